# revision 35
# baseline (speedup 1.0000x reference)
"""Trainium2 Bass kernel for AffinityMatrixConstructLayer (v5).

Math (matching the reference's index conventions):
  weight W[b, a] = softplusrelu( sum_d ef1[b,d]*ce[d]*ef2[a,d] )  (b: g2 edge)
  M[(i2,i1),(k2,k1)] = sum_{b: h2(b)=i2, t2(b)=k2} sum_{a: h1(a)=i1, t1(a)=k1}
                       W[b, a]  +  diag(Mp[i2, i1])
  cn/ce = tanh(Wn/We @ gw + bias)

Design notes:
 - Collectives cost ~55us end-to-end on this stack (measured), so the
   coeff matvec is replicated per core. W streams in FOUR 1MB rho-blocks
   (We rows 0:512, Wn 0:512, We 512:1024, Wn 512:1024) so the matvec,
   tanh and even the Meb GEMM chunks pipeline underneath the 4MB DMA:
   per block, 8 PE matmuls (lhsT = gw chunk [128,1], rhs = W^T chunk,
   N=512) accumulate a [1,512] psum row; 4 PE transposes of [1,128]
   slices + DVE copies build coeff_pre columns; exp-form tanh follows
   per block.  8 dummy matmuls warm the PE HAM clock during the initial
   DMA wait.
 - Each core needs only the <=48 graph-2 edges with head in its 6 block
   rows: one-hot SelT [192,48] compacts Meb -> McT (N=48); then
   Pc = McT.T @ B1 (K=192, N=2304) and out_pair = S2c.T @ Pc x3 pairs.
 - The Mp diagonal is folded into the final GEMM: the an GEMM computes
   the 6 owned Mp rows at partitions 48:54 (host stages x1 there), the
   softplus'd rows are strided-scattered into pc_sb rows 48:54, and
   host-built S2c rows 48:54 select them -- no cross-partition bounce.
 - All big GEMM operands bf16 (f32 PSUM); output written bf16, host
   upcasts (tolerance 2e-2).  ACT funcs stay in one table set.
 - B1 one-hot is built on gpsimd/DVE during the W stream; per-i2 k2
   rotation baked into S2c puts the diagonal at k2rot=0; host un-rotates.
"""

import sys

for _p in ("/opt/trn_rl_repo", "/root/.axon_site/_ro/trn_rl_repo"):
    if _p not in sys.path:
        sys.path.insert(0, _p)

import numpy as np
import ml_dtypes

import concourse.bass as bass
import concourse.mybir as mybir
from concourse.tile import TileContext
from concourse.masks import make_identity
from concourse.bass_utils import run_bass_kernel_spmd

F32 = mybir.dt.float32
BF16 = mybir.dt.bfloat16
I32 = mybir.dt.int32
AF = mybir.ActivationFunctionType
ALU = mybir.AluOpType

N_CORES = 8
N = 48          # nodes per graph
E = 192         # edges per graph
D = 1024        # feature dim
KC = 8          # contraction chunks of 128
I2P = N // N_CORES          # 6 block-rows per core
JC = 48         # compacted graph-2 edge capacity per core
DROW = 64       # diag rows base partition (32-aligned engine access)
KF = DROW + I2P  # final-GEMM K: 48 edges + pad + 6 diag rows
ROWS = I2P * N              # 288 valid output rows per core
OROWS = 3 * 112             # padded device output rows
COLS = N * N                # 2304
NT = [(t * 512, min(COLS, (t + 1) * 512)) for t in range((COLS + 511) // 512)]
# W stream order: block -> (source, row-tile); e-blocks early so the Meb
# path runs under the DMA stream; coeff col of (block, t) = CBASE[B] + t
WBLK = [("e", 0), ("n", 0), ("e", 1), ("n", 1)]
CBASE = [8, 0, 12, 4]

_CACHE: dict = {}
LAST_RESULTS = None


def _split_multiwaits(nc):
    """This walrus build encodes at most one sync-wait per instruction.
    Move extra waits onto injected single-wait drains on the same engine
    (engine queues execute in order, so semantics are preserved)."""
    for f in nc.m.functions:
        for blk in f.blocks:
            out = []
            for inst in blk.instructions:
                si = getattr(inst, "sync_info", None)
                if si is not None and si.on_wait and len(si.on_wait) > 1:
                    waits = list(si.on_wait)
                    for w in waits[:-1]:
                        d = mybir.InstDrain(
                            name=nc.get_next_instruction_name(),
                            ins=[], outs=[], bass_is_fusable=False)
                        d.engine = inst.engine
                        d.sync_info = mybir.SyncInfo(on_wait=[w], on_update=[])
                        out.append(d)
                    si.on_wait = waits[-1:]
                out.append(inst)
            try:
                blk.instructions[:] = out
            except TypeError:
                blk.instructions = out


def _build() -> bass.Bass:
    if "nc" in _CACHE:
        return _CACHE["nc"]
    nc = bass.Bass(trn_type="TRN2", num_devices=N_CORES)

    d_wb = nc.dram_tensor("wb", [128, 4 * KC * 512], BF16,
                          kind="ExternalInput")
    d_gwc = nc.dram_tensor("gwc", [128, KC], BF16, kind="ExternalInput")
    d_bnbe = nc.dram_tensor("bnbe", [128, 16], F32, kind="ExternalInput")
    d_ei1 = nc.dram_tensor("ei1", [2, E], I32, kind="ExternalInput")
    d_x1tp = nc.dram_tensor("x1tp", [128, KC * KF], BF16,
                            kind="ExternalInput")
    d_x2tp = nc.dram_tensor("x2tp", [128, KC * N], BF16, kind="ExternalInput")
    d_ef1tp = nc.dram_tensor("ef1tp", [128, KC * E], BF16,
                             kind="ExternalInput")
    d_ef2tp = nc.dram_tensor("ef2tp", [128, KC * E], BF16,
                             kind="ExternalInput")
    d_selth = nc.dram_tensor("selth", [E, JC], BF16, kind="ExternalInput")
    d_s2ch = nc.dram_tensor("s2ch", [KF, 3 * 112], BF16, kind="ExternalInput")
    d_out = nc.dram_tensor("out", [OROWS, COLS], BF16, kind="ExternalOutput")

    with TileContext(nc) as tc:
        with (
            tc.tile_pool(name="const", bufs=1) as cpool,
            tc.tile_pool(name="scratch", bufs=2) as spool,
            tc.tile_pool(name="orow", bufs=3) as opool,
            tc.tile_pool(name="pmv", bufs=1, space="PSUM") as pmv,
            tc.tile_pool(name="pbig", bufs=1, space="PSUM") as pbig,
            tc.tile_pool(name="ppc", bufs=2, space="PSUM") as ppc,
            tc.tile_pool(name="pfin", bufs=2, space="PSUM") as pfin,
        ):
            FLOOR = tc.tile_wait_until   # scheduler virtual-time floor (ms)

            # ---------- W stream: sync carries 3 halves, gpsimd 5 ----------
            gwc = cpool.tile([128, KC], BF16, tag="gwc", name="gwc")
            nc.sync.dma_start(out=gwc, in_=d_gwc[:, :])
            bnbe = cpool.tile([128, 16], F32, tag="bnbe", name="bnbe")
            nc.sync.dma_start(out=bnbe, in_=d_bnbe[:, :])
            BW = KC * 512
            HWW = BW // 2
            wb = cpool.tile([128, 4 * BW], BF16, tag="wb", name="wb")

            # gpsimd queue: edge vectors, early bulk, then its W halves
            ident = cpool.tile([128, 128], F32, tag="ident", name="ident")
            make_identity(nc, ident)
            iota48 = cpool.tile([128, N], F32, tag="iota48", name="iota48")
            nc.gpsimd.iota(iota48, pattern=[[1, N]], base=0,
                           channel_multiplier=0,
                           allow_small_or_imprecise_dtypes=True)
            ev_tiles = []
            for lo, hi in ((0, 128), (128, 192)):
                t = cpool.tile([hi - lo, 2], F32, tag=f"ev{lo}",
                               name=f"ev{lo}")
                nc.gpsimd.dma_start(
                    out=t, in_=d_ei1[:, lo:hi].rearrange("a b -> b a"))
                ev_tiles.append(t)
            ef1tp = cpool.tile([128, KC * E], BF16, tag="ef1tp", name="ef1tp")
            nc.gpsimd.dma_start(out=ef1tp, in_=d_ef1tp[:, :])
            ef2tp = cpool.tile([128, KC * E], BF16, tag="ef2tp", name="ef2tp")
            nc.gpsimd.dma_start(out=ef2tp, in_=d_ef2tp[:, :])
            selth0 = cpool.tile([128, JC], BF16, tag="selth0", name="selth0")
            nc.gpsimd.dma_start(out=selth0, in_=d_selth[0:128, :])
            selth1 = cpool.tile([64, JC], BF16, tag="selth1", name="selth1")
            nc.gpsimd.dma_start(out=selth1, in_=d_selth[128:192, :])
            for h in (0, 1, 2, 4, 5):
                nc.gpsimd.dma_start(out=wb[:, h * HWW:(h + 1) * HWW],
                                    in_=d_wb[:, h * HWW:(h + 1) * HWW])
            for h in (3, 6, 7):
                nc.sync.dma_start(out=wb[:, h * HWW:(h + 1) * HWW],
                                  in_=d_wb[:, h * HWW:(h + 1) * HWW])
            x1tp = cpool.tile([128, KC * KF], BF16, tag="x1tp", name="x1tp")
            nc.gpsimd.dma_start(out=x1tp, in_=d_x1tp[:, :])
            x2tp = cpool.tile([128, KC * N], BF16, tag="x2tp", name="x2tp")
            nc.gpsimd.dma_start(out=x2tp, in_=d_x2tp[:, :])
            s2ch = cpool.tile([KF, 3 * 112], BF16, tag="s2ch", name="s2ch")
            nc.gpsimd.dma_start(out=s2ch, in_=d_s2ch[:, :])

            # pc_sb zero-fill on idle gpsimd
            pc_sb = cpool.tile([KF, COLS], BF16, tag="pc_sb", name="pc_sb")
            nc.gpsimd.memset(pc_sb[32:64, :], 0.0)
            nc.gpsimd.memset(pc_sb[64:KF, :], 0.0)

            # ---------- B1 one-hot built on DVE during the W stream -------
            def incid(col, tag):
                tiles = []
                for ci, p in ((0, 128), (1, 64)):
                    ev = ev_tiles[ci][:, col:col + 1]
                    t = cpool.tile([p, N], BF16, tag=f"{tag}{ci}",
                                   name=f"{tag}{ci}")
                    nc.vector.tensor_tensor(t, iota48[0:p, :],
                                            ev.broadcast_to((p, N)),
                                            ALU.is_equal)
                    tiles.append(t)
                return tiles

            G1T = incid(0, "G1T")
            H1T = incid(1, "H1T")
            b1 = []
            for ci, p in ((0, 128), (1, 64)):
                bt = cpool.tile([p, COLS], BF16, tag=f"b1{ci}",
                                name=f"b1{ci}")
                b1.append(bt)

            def b1_mul(ci):
                p = 128 if ci == 0 else 64
                nc.vector.tensor_mul(
                    b1[ci].rearrange("p (a b) -> p a b", b=N),
                    H1T[ci].unsqueeze(1).broadcast_to((p, N, N)),
                    G1T[ci].unsqueeze(2).broadcast_to((p, N, N)))

            # ---------- PE warmup during the first DMA wait ----------
            dsrc = cpool.tile([128, 512], BF16, tag="dsrc", name="dsrc")
            nc.vector.memset(dsrc, 0.0)
            _wi = [0]

            def warm(n):
                for _ in range(n):
                    dpp = pfin.tile([112, 512], F32, tag="fin",
                                    name=f"warm{_wi[0]}")
                    _wi[0] += 1
                    nc.tensor.matmul(dpp, dsrc[:, 0:112], dsrc,
                                     start=True, stop=True)

            warm(8)

            # ---------- per-block matvec -> coeff columns ----------
            mv_sb = [cpool.tile([1, 512], F32, tag=f"mvsb{B}",
                                name=f"mvsb{B}") for B in range(4)]
            cpre = cpool.tile([128, 16], F32, tag="cpre", name="cpre")
            coeff = cpool.tile([128, 16], F32, tag="coeff", name="coeff")
            aef1 = cpool.tile([128, KC * E], BF16, tag="aef1", name="aef1")
            a1 = cpool.tile([128, KC * KF], BF16, tag="a1", name="a1")
            meb0 = pbig.tile([128, E], F32, tag="pB", name="meb0")
            meb1 = pbig.tile([64, E], F32, tag="pC", name="meb1")
            an = pmv.tile([KF, N], F32, tag="pAn", name="an")

            def block_tail(B):
                """mv psum row -> coeff cols CBASE[B]..CBASE[B]+4."""
                base = CBASE[B]
                pt = ppc.tile([128, 16], F32, tag="pc", name=f"pt{B}")
                for t in range(4):
                    nc.tensor.transpose(
                        pt[:, base + t:base + t + 1],
                        mv_sb[B][:, t * 128:(t + 1) * 128],
                        ident[0:1, 0:1])
                cs = slice(base, base + 4)
                nc.vector.tensor_copy(cpre[:, cs], pt[:, cs])
                z2 = spool.tile([128, 4], F32, tag="z2", name=f"z2{B}")
                nc.vector.scalar_tensor_tensor(out=z2, in0=cpre[:, cs],
                                               scalar=2.0, in1=bnbe[:, cs],
                                               op0=ALU.mult, op1=ALU.add)
                et = spool.tile([128, 4], F32, tag="et", name=f"et{B}")
                nc.scalar.activation(et, z2, AF.Exp)
                nc.vector.tensor_scalar_add(et, et, 1.0)
                rt = spool.tile([128, 4], F32, tag="rt", name=f"rt{B}")
                nc.vector.reciprocal(rt, et)
                nc.vector.tensor_scalar(coeff[:, cs], rt, -2.0, 1.0,
                                        ALU.mult, ALU.add)

            def mv_block(B):
                mvb = pmv.tile([1, 512], F32, tag="pA", name=f"mvb{B}")
                for k in range(KC):
                    if k == 4:
                        warm(1)
                    nc.tensor.matmul(
                        mvb, gwc[:, k:k + 1],
                        wb[:, B * BW + k * 512:B * BW + (k + 1) * 512],
                        start=(k == 0), stop=(k == KC - 1))
                nc.scalar.copy(mv_sb[B], mvb)
                block_tail(B)
                warm(1)

            def meb_chunks(ks):
                for k in ks:
                    ke = slice(k * E, (k + 1) * E)
                    nc.scalar.activation(aef1[:, ke], ef1tp[:, ke],
                                         AF.Copy,
                                         scale=coeff[:, 8 + k:9 + k])
                    nc.tensor.matmul(meb0, aef1[:, k * E:k * E + 128],
                                     ef2tp[:, ke],
                                     start=(k == 0), stop=(k == KC - 1))
                    nc.tensor.matmul(meb1, aef1[:, k * E + 128:(k + 1) * E],
                                     ef2tp[:, ke],
                                     start=(k == 0), stop=(k == KC - 1))

            def an_chunks(ks):
                for k in ks:
                    kn = slice(k * KF, (k + 1) * KF)
                    nc.scalar.activation(a1[:, kn], x1tp[:, kn], AF.Copy,
                                         scale=coeff[:, k:k + 1])
                    nc.tensor.matmul(an, a1[:, kn],
                                     x2tp[:, k * N:(k + 1) * N],
                                     start=(k == 0), stop=(k == KC - 1))

            with FLOOR(0.0115):
                mv_block(0)                  # We rows 0:512
            with FLOOR(0.0118):
                b1_mul(0)
            with FLOOR(0.0130):
                meb_chunks(range(0, 4))
            with FLOOR(0.0125):
                mv_block(1)                  # Wn rows 0:512
            with FLOOR(0.0135):
                an_chunks(range(0, 4))
            with FLOOR(0.0150):
                b1_mul(1)
            with FLOOR(0.0170):
                mv_block(2)                  # We rows 512:1024
            with FLOOR(0.0176):
                meb_chunks(range(4, 8))

            # relu(softplus(x) - 0.5): softplus = relu(x) + ln(1+exp(-|x|))
            # All intermediate tiles sliced at the source partition base
            # (compute engines are lane-locked).
            def softplus_relu(src_ap, out_ap, pbase=0):
                p, w = src_ap.shape[0], src_ap.shape[1]
                pe = pbase + p

                def tmp(nm):
                    t = spool.tile([pe, w], F32, tag=f"sp_{nm}",
                                   name=f"sp_{nm}")
                    return t[pbase:pe, :]

                ab_t = tmp("ab")
                nc.scalar.activation(ab_t, src_ap, AF.Abs)
                ex = tmp("ex")
                nc.scalar.activation(ex, ab_t, AF.Exp, scale=-1.0)
                ln = tmp("ln")
                nc.scalar.activation(ln, ex, AF.Ln, bias=1.0)
                rl = tmp("rl")
                nc.scalar.activation(rl, src_ap, AF.Relu)
                pre = tmp("pre")
                nc.vector.scalar_tensor_tensor(out=pre, in0=rl, scalar=-0.5,
                                               in1=ln, op0=ALU.add,
                                               op1=ALU.add)
                nc.vector.tensor_scalar_max(out_ap, pre, 0.0)

            mebs0 = cpool.tile([128, E], BF16, tag="mebs0", name="mebs0")
            mebs1 = cpool.tile([64, E], BF16, tag="mebs1", name="mebs1")
            with FLOOR(0.0185):
                softplus_relu(meb0, mebs0)
                softplus_relu(meb1, mebs1)

            # ---------- McT[a, jc] = Meb[glob(jc), a] (edge compaction) ---
            mct0 = pbig.tile([128, JC], F32, tag="pB", name="mct0")
            mct1 = pbig.tile([64, JC], F32, tag="pC", name="mct1")
            mcts0 = cpool.tile([128, JC], BF16, tag="mcts0", name="mcts0")
            mcts1 = cpool.tile([64, JC], BF16, tag="mcts1", name="mcts1")
            with FLOOR(0.0195):
                nc.tensor.matmul(mct0, mebs0[:, 0:128], selth0,
                                 start=True, stop=False)
                nc.tensor.matmul(mct0, mebs1[:, 0:128], selth1,
                                 start=False, stop=True)
                nc.tensor.matmul(mct1, mebs0[:, 128:192], selth0,
                                 start=True, stop=False)
                nc.tensor.matmul(mct1, mebs1[:, 128:192], selth1,
                                 start=False, stop=True)
                nc.scalar.copy(mcts0, mct0)
                nc.vector.tensor_copy(mcts1, mct1)

            # ---------- Pc rows 0:48 = McT.T @ B1; rows 64:70 = diag ------
            def pc_tiles(tis):
                for ti in tis:
                    t0, t1 = NT[ti]
                    w = t1 - t0
                    pp = ppc.tile([JC, 512], F32, tag="pc", name="pp")
                    nc.tensor.matmul(pp[:, 0:w], mcts0, b1[0][:, t0:t1],
                                     start=True, stop=False)
                    nc.tensor.matmul(pp[:, 0:w], mcts1, b1[1][:, t0:t1],
                                     start=False, stop=True)
                    if ti % 2 == 0:
                        nc.vector.tensor_copy(pc_sb[0:JC, t0:t1], pp[:, 0:w])
                    else:
                        nc.scalar.copy(pc_sb[0:JC, t0:t1], pp[:, 0:w])

            with FLOOR(0.0198):
                pc_tiles(range(0, 3))
            with FLOOR(0.0186):
                mv_block(3)                  # Wn rows 512:1024
            with FLOOR(0.0200):
                an_chunks(range(4, 8))
            with FLOOR(0.0207):
                pc_tiles(range(3, 5))
            # Mp rows live at an[64:70]; scatter onto the diagonal stride
            msel = cpool.tile([KF, N], BF16, tag="msel", name="msel")
            with FLOOR(0.0205):
                softplus_relu(an[DROW:KF, :], msel[DROW:KF, :], pbase=DROW)
                nc.vector.tensor_copy(pc_sb[DROW:KF, 0:COLS:N + 1],
                                      msel[DROW:KF, :])

            # ---------- final: out_pair = S2c.T @ Pc (diag included) ------
            with FLOOR(0.0215):
                for pa in range(I2P // 2):
                    orow = opool.tile([112, COLS], BF16, tag="orow",
                                      name="orow")
                    for ti, (t0, t1) in enumerate(NT):
                        w = t1 - t0
                        fp = pfin.tile([112, 512], F32, tag="fin", name="fp")
                        nc.tensor.matmul(fp[:, 0:w],
                                         s2ch[:, pa * 112:(pa + 1) * 112],
                                         pc_sb[:, t0:t1],
                                         start=True, stop=True)
                        if ti % 2 == 0:
                            nc.vector.tensor_copy(orow[:, t0:t1], fp[:, 0:w])
                        else:
                            nc.scalar.copy(orow[:, t0:t1], fp[:, 0:w])
                    nc.sync.dma_start(out=d_out[pa * 112:(pa + 1) * 112, :],
                                      in_=orow)

    _split_multiwaits(nc)
    _CACHE["nc"] = nc
    return nc


def _make_in_maps(a):
    bf = ml_dtypes.bfloat16
    gw = a["global_weight"].astype(np.float32)
    gwc = np.ascontiguousarray(gw.reshape(KC, 128).T).astype(bf)

    def wblock(Wfull, r):  # rows 512r:512r+512 -> [128, KC*512] lhs-moving
        sl = Wfull[512 * r:512 * (r + 1), :].astype(np.float32)
        t = sl.T.reshape(KC, 128, 512).transpose(1, 0, 2)
        return t.reshape(128, KC * 512)

    wn, we = a["Wn"], a["We"]
    wbs = {("n", r): wblock(wn, r) for r in range(2)}
    wbs.update({("e", r): wblock(we, r) for r in range(2)})
    wb = np.concatenate([wbs[b] for b in WBLK], axis=1)
    wb = np.ascontiguousarray(wb).astype(bf)

    bnbe = np.zeros((128, 16), np.float32)
    for j in range(16):
        b = a["bn"] if j < 8 else a["be"]
        bnbe[:, j] = 2.0 * b[128 * (j % 8):128 * (j % 8) + 128]

    def chunked(x):  # [n, 1024] -> [128, KC*n] bf16, chunk k = feats 128k+p
        t = x.T.astype(np.float32).reshape(KC, 128, -1).transpose(1, 0, 2)
        return np.ascontiguousarray(t.reshape(128, -1)).astype(bf)

    x2tp = chunked(a["x2"])
    ef1tp = chunked(a["ef1"])
    ef2tp = chunked(a["ef2"])

    ei1 = a["edge_index1"].astype(np.int32)
    ei2 = a["edge_index2"].astype(np.int64)

    in_maps = []
    for c in range(N_CORES):
        edges = np.where(ei2[0] // I2P == c)[0]
        assert len(edges) <= JC, f"core {c}: {len(edges)} edges > JC={JC}"
        selth = np.zeros((E, JC), np.float32)
        selth[edges, np.arange(len(edges))] = 1.0
        s2ch = np.zeros((KF, 3 * 112), np.float32)
        for j, e in enumerate(edges):
            i2g = int(ei2[0, e])
            i2l = i2g - I2P * c
            k2r = (int(ei2[1, e]) - i2g) % N
            s2ch[j, (i2l // 2) * 112 + 64 * (i2l % 2) + k2r] = 1.0
        for i2l in range(I2P):  # diag rows select pc_sb rows 64:70
            s2ch[DROW + i2l, (i2l // 2) * 112 + 64 * (i2l % 2)] = 1.0
        # x1 staged into columns 64:70 of the an operand (owned rows only)
        x1own = a["x1"][I2P * c:I2P * (c + 1), :]        # [6, 1024]
        t = x1own.T.astype(np.float32).reshape(KC, 128, I2P)
        x1tp = np.zeros((128, KC * KF), np.float32)
        for k in range(KC):
            x1tp[:, k * KF + DROW:(k + 1) * KF] = t[k]
        in_maps.append({
            "wb": wb,
            "gwc": gwc,
            "bnbe": bnbe,
            "ei1": ei1,
            "x1tp": x1tp.astype(bf),
            "x2tp": x2tp,
            "ef1tp": ef1tp,
            "ef2tp": ef2tp,
            "selth": selth.astype(bf),
            "s2ch": s2ch.astype(bf),
        })
    return in_maps


def kernel(**inputs) -> np.ndarray:
    global LAST_RESULTS
    nc = _build()
    a = {k: np.ascontiguousarray(np.asarray(v)) for k, v in inputs.items()}
    in_maps = _make_in_maps(a)
    res = run_bass_kernel_spmd(nc, in_maps, core_ids=list(range(N_CORES)))
    LAST_RESULTS = res

    parts = []
    for c in range(N_CORES):
        # device rows are [pa][0:48] = i2l 2pa, [64:112] = i2l 2pa+1, each
        # [k2rot, (i1, k1)] with k2g = (k2rot + i2l + 6c) mod 48
        o = np.asarray(res.results[c]["out"]).astype(np.float32)
        o = o.reshape(3, 112, COLS)
        o = np.stack([o[i2l // 2, 64 * (i2l % 2):64 * (i2l % 2) + N, :]
                      for i2l in range(I2P)])    # [6, k2rot, 2304]
        o = o.reshape(I2P, N, N, N).transpose(0, 2, 1, 3)
        o = np.stack([np.roll(o[i], i + I2P * c, axis=1)
                      for i in range(I2P)])
        parts.append(o.reshape(ROWS, COLS))
    return np.concatenate(parts, axis=0).astype(np.float32)


if __name__ == "__main__":
    _build()
    print("build OK")


# revision 36
# speedup vs baseline: 1.2116x; 1.2116x over previous
"""Trainium2 Bass kernel for AffinityMatrixConstructLayer (v5).

Math (matching the reference's index conventions):
  weight W[b, a] = softplusrelu( sum_d ef1[b,d]*ce[d]*ef2[a,d] )  (b: g2 edge)
  M[(i2,i1),(k2,k1)] = sum_{b: h2(b)=i2, t2(b)=k2} sum_{a: h1(a)=i1, t1(a)=k1}
                       W[b, a]  +  diag(Mp[i2, i1])
  cn/ce = tanh(Wn/We @ gw + bias)

Design notes:
 - Collectives cost ~55us end-to-end on this stack (measured), so the
   coeff matvec is replicated per core. W streams in FOUR 1MB rho-blocks
   (We rows 0:512, Wn 0:512, We 512:1024, Wn 512:1024) so the matvec,
   tanh and even the Meb GEMM chunks pipeline underneath the 4MB DMA:
   per block, 8 PE matmuls (lhsT = gw chunk [128,1], rhs = W^T chunk,
   N=512) accumulate a [1,512] psum row; 4 PE transposes of [1,128]
   slices + DVE copies build coeff_pre columns; exp-form tanh follows
   per block.  8 dummy matmuls warm the PE HAM clock during the initial
   DMA wait.
 - Each core needs only the <=48 graph-2 edges with head in its 6 block
   rows: one-hot SelT [192,48] compacts Meb -> McT (N=48); then
   Pc = McT.T @ B1 (K=192, N=2304) and out_pair = S2c.T @ Pc x3 pairs.
 - The Mp diagonal is folded into the final GEMM: the an GEMM computes
   the 6 owned Mp rows at partitions 48:54 (host stages x1 there), the
   softplus'd rows are strided-scattered into pc_sb rows 48:54, and
   host-built S2c rows 48:54 select them -- no cross-partition bounce.
 - All big GEMM operands bf16 (f32 PSUM); output written bf16, host
   upcasts (tolerance 2e-2).  ACT funcs stay in one table set.
 - B1 one-hot is built on gpsimd/DVE during the W stream; per-i2 k2
   rotation baked into S2c puts the diagonal at k2rot=0; host un-rotates.
"""

import sys

for _p in ("/opt/trn_rl_repo", "/root/.axon_site/_ro/trn_rl_repo"):
    if _p not in sys.path:
        sys.path.insert(0, _p)

import numpy as np
import ml_dtypes

import concourse.bass as bass
import concourse.mybir as mybir
from concourse.tile import TileContext
from concourse.masks import make_identity
from concourse.bass_utils import run_bass_kernel_spmd

F32 = mybir.dt.float32
BF16 = mybir.dt.bfloat16
I32 = mybir.dt.int32
AF = mybir.ActivationFunctionType
ALU = mybir.AluOpType

N_CORES = 8
N = 48          # nodes per graph
E = 192         # edges per graph
D = 1024        # feature dim
KC = 8          # contraction chunks of 128
I2P = N // N_CORES          # 6 block-rows per core
JC = 48         # compacted graph-2 edge capacity per core
DROW = 64       # diag rows base partition (32-aligned engine access)
KF = DROW + I2P  # final-GEMM K: 48 edges + pad + 6 diag rows
ROWS = I2P * N              # 288 valid output rows per core
OROWS = 3 * 112             # padded device output rows
COLS = N * N                # 2304
NT = [(t * 512, min(COLS, (t + 1) * 512)) for t in range((COLS + 511) // 512)]
# W stream order: block -> (source, row-tile); e-blocks early so the Meb
# path runs under the DMA stream; coeff col of (block, t) = CBASE[B] + t
WBLK = [("e", 0), ("n", 0), ("e", 1), ("n", 1)]
CBASE = [8, 0, 12, 4]

_CACHE: dict = {}
LAST_RESULTS = None


def _split_multiwaits(nc):
    """This walrus build encodes at most one sync-wait per instruction.
    Move extra waits onto injected single-wait drains on the same engine
    (engine queues execute in order, so semantics are preserved)."""
    for f in nc.m.functions:
        for blk in f.blocks:
            out = []
            for inst in blk.instructions:
                si = getattr(inst, "sync_info", None)
                if si is not None and si.on_wait and len(si.on_wait) > 1:
                    waits = list(si.on_wait)
                    for w in waits[:-1]:
                        d = mybir.InstDrain(
                            name=nc.get_next_instruction_name(),
                            ins=[], outs=[], bass_is_fusable=False)
                        d.engine = inst.engine
                        d.sync_info = mybir.SyncInfo(on_wait=[w], on_update=[])
                        out.append(d)
                    si.on_wait = waits[-1:]
                out.append(inst)
            try:
                blk.instructions[:] = out
            except TypeError:
                blk.instructions = out


def _build() -> bass.Bass:
    if "nc" in _CACHE:
        return _CACHE["nc"]
    nc = bass.Bass(trn_type="TRN2", num_devices=N_CORES)

    d_wb = nc.dram_tensor("wb", [128, 4 * KC * 512], BF16,
                          kind="ExternalInput")
    d_gwc = nc.dram_tensor("gwc", [128, KC], BF16, kind="ExternalInput")
    d_bnbe = nc.dram_tensor("bnbe", [128, 16], F32, kind="ExternalInput")
    d_ei1f = nc.dram_tensor("ei1f", [E, 2], F32, kind="ExternalInput")
    d_x1tp = nc.dram_tensor("x1tp", [128, KC * KF], BF16,
                            kind="ExternalInput")
    d_x2tp = nc.dram_tensor("x2tp", [128, KC * N], BF16, kind="ExternalInput")
    d_ef1tp = nc.dram_tensor("ef1tp", [128, KC * E], BF16,
                             kind="ExternalInput")
    d_ef2tp = nc.dram_tensor("ef2tp", [128, KC * E], BF16,
                             kind="ExternalInput")
    d_selth = nc.dram_tensor("selth", [E, JC], BF16, kind="ExternalInput")
    d_s2ch = nc.dram_tensor("s2ch", [KF, 3 * 112], BF16, kind="ExternalInput")
    d_out = nc.dram_tensor("out", [OROWS, COLS], BF16, kind="ExternalOutput")

    with TileContext(nc) as tc:
        with (
            tc.tile_pool(name="const", bufs=1) as cpool,
            tc.tile_pool(name="scratch", bufs=2) as spool,
            tc.tile_pool(name="orow", bufs=3) as opool,
            tc.tile_pool(name="pmv", bufs=1, space="PSUM") as pmv,
            tc.tile_pool(name="pbig", bufs=1, space="PSUM") as pbig,
            tc.tile_pool(name="ppc", bufs=2, space="PSUM") as ppc,
            tc.tile_pool(name="pfin", bufs=2, space="PSUM") as pfin,
        ):
            FLOOR = tc.tile_wait_until   # scheduler virtual-time floor (ms)

            # ---------- front DMA: bandwidth-balanced across the sync
            # (HWDGE, ~150GB/s) and gpsimd (SWDGE, ~250GB/s) queues, in
            # order of real need: e0 W halves + ef chunk-halves first
            # (the Meb critical path), then e1, then the n blocks. ------
            gwc = cpool.tile([128, KC], BF16, tag="gwc", name="gwc")
            nc.sync.dma_start(out=gwc, in_=d_gwc[:, :])
            bnbe = cpool.tile([128, 16], F32, tag="bnbe", name="bnbe")
            nc.sync.dma_start(out=bnbe, in_=d_bnbe[:, :])
            ev_tiles = []
            for lo, hi in ((0, 128), (128, 192)):
                t = cpool.tile([hi - lo, 2], F32, tag=f"ev{lo}",
                               name=f"ev{lo}")
                nc.sync.dma_start(out=t, in_=d_ei1f[lo:hi, :])
                ev_tiles.append(t)
            ident = cpool.tile([128, 128], F32, tag="ident", name="ident")
            make_identity(nc, ident)
            iota48 = cpool.tile([128, N], F32, tag="iota48", name="iota48")
            nc.gpsimd.iota(iota48, pattern=[[1, N]], base=0,
                           channel_multiplier=0,
                           allow_small_or_imprecise_dtypes=True)

            BW = KC * 512
            HWW = BW // 2
            wb = cpool.tile([128, 4 * BW], BF16, tag="wb", name="wb")
            ef1tp = cpool.tile([128, KC * E], BF16, tag="ef1tp", name="ef1tp")
            ef2tp = cpool.tile([128, KC * E], BF16, tag="ef2tp", name="ef2tp")
            x1tp = cpool.tile([128, KC * KF], BF16, tag="x1tp", name="x1tp")
            x2tp = cpool.tile([128, KC * N], BF16, tag="x2tp", name="x2tp")
            selth0 = cpool.tile([128, JC], BF16, tag="selth0", name="selth0")
            selth1 = cpool.tile([64, JC], BF16, tag="selth1", name="selth1")
            s2ch = cpool.tile([KF, 3 * 112], BF16, tag="s2ch", name="s2ch")

            def wh(h, eng):
                eng.dma_start(out=wb[:, h * HWW:(h + 1) * HWW],
                              in_=d_wb[:, h * HWW:(h + 1) * HWW])

            EH = 4 * E
            wh(0, nc.gpsimd)                               # e0 lo
            wh(1, nc.sync)                                 # e0 hi
            nc.gpsimd.dma_start(out=ef1tp[:, 0:EH], in_=d_ef1tp[:, 0:EH])
            nc.sync.dma_start(out=ef2tp[:, 0:EH], in_=d_ef2tp[:, 0:EH])
            wh(4, nc.gpsimd)                               # e1 lo
            wh(5, nc.sync)                                 # e1 hi
            nc.gpsimd.dma_start(out=ef1tp[:, EH:2 * EH],
                                in_=d_ef1tp[:, EH:2 * EH])
            nc.sync.dma_start(out=ef2tp[:, EH:2 * EH],
                              in_=d_ef2tp[:, EH:2 * EH])
            wh(2, nc.gpsimd)                               # n0 lo
            wh(3, nc.sync)                                 # n0 hi
            wh(6, nc.gpsimd)                               # n1 lo
            wh(7, nc.sync)                                 # n1 hi
            nc.gpsimd.dma_start(out=x1tp, in_=d_x1tp[:, :])
            nc.gpsimd.dma_start(out=x2tp, in_=d_x2tp[:, :])
            nc.gpsimd.dma_start(out=selth0, in_=d_selth[0:128, :])
            nc.gpsimd.dma_start(out=selth1, in_=d_selth[128:192, :])
            nc.gpsimd.dma_start(out=s2ch, in_=d_s2ch[:, :])

            # pc_sb zero-fill on idle gpsimd
            pc_sb = cpool.tile([KF, COLS], BF16, tag="pc_sb", name="pc_sb")
            nc.gpsimd.memset(pc_sb[32:64, :], 0.0)
            nc.gpsimd.memset(pc_sb[64:KF, :], 0.0)

            # ---------- B1 one-hot built on DVE during the W stream -------
            def incid(col, tag):
                tiles = []
                for ci, p in ((0, 128), (1, 64)):
                    ev = ev_tiles[ci][:, col:col + 1]
                    t = cpool.tile([p, N], BF16, tag=f"{tag}{ci}",
                                   name=f"{tag}{ci}")
                    nc.vector.tensor_tensor(t, iota48[0:p, :],
                                            ev.broadcast_to((p, N)),
                                            ALU.is_equal)
                    tiles.append(t)
                return tiles

            G1T = incid(0, "G1T")
            H1T = incid(1, "H1T")
            b1 = []
            for ci, p in ((0, 128), (1, 64)):
                bt = cpool.tile([p, COLS], BF16, tag=f"b1{ci}",
                                name=f"b1{ci}")
                b1.append(bt)

            def b1_mul(ci):
                p = 128 if ci == 0 else 64
                nc.vector.tensor_mul(
                    b1[ci].rearrange("p (a b) -> p a b", b=N),
                    H1T[ci].unsqueeze(1).broadcast_to((p, N, N)),
                    G1T[ci].unsqueeze(2).broadcast_to((p, N, N)))

            # ---------- PE warmup during the first DMA wait ----------
            dsrc = cpool.tile([128, 512], BF16, tag="dsrc", name="dsrc")
            nc.vector.memset(dsrc, 0.0)
            _wi = [0]

            def warm(n):
                for _ in range(n):
                    dpp = pfin.tile([112, 512], F32, tag="fin",
                                    name=f"warm{_wi[0]}")
                    _wi[0] += 1
                    nc.tensor.matmul(dpp, dsrc[:, 0:112], dsrc,
                                     start=True, stop=True)

            warm(8)

            # ---------- per-block matvec -> coeff columns ----------
            mv_sb = [cpool.tile([1, 512], F32, tag=f"mvsb{B}",
                                name=f"mvsb{B}") for B in range(4)]
            cpre = cpool.tile([128, 16], F32, tag="cpre", name="cpre")
            coeff = cpool.tile([128, 16], F32, tag="coeff", name="coeff")
            aef1 = cpool.tile([128, KC * E], BF16, tag="aef1", name="aef1")
            a1 = cpool.tile([128, KC * KF], BF16, tag="a1", name="a1")
            meb0 = pbig.tile([128, E], F32, tag="pB", name="meb0")
            meb1 = pbig.tile([64, E], F32, tag="pC", name="meb1")
            an = pmv.tile([KF, N], F32, tag="pAn", name="an")

            def block_tail(B):
                """mv psum row -> coeff cols CBASE[B]..CBASE[B]+4."""
                base = CBASE[B]
                pt = ppc.tile([128, 16], F32, tag="pc", name=f"pt{B}")
                for t in range(4):
                    nc.tensor.transpose(
                        pt[:, base + t:base + t + 1],
                        mv_sb[B][:, t * 128:(t + 1) * 128],
                        ident[0:1, 0:1])
                cs = slice(base, base + 4)
                nc.vector.tensor_copy(cpre[:, cs], pt[:, cs])
                z2 = spool.tile([128, 4], F32, tag="z2", name=f"z2{B}")
                nc.vector.scalar_tensor_tensor(out=z2, in0=cpre[:, cs],
                                               scalar=2.0, in1=bnbe[:, cs],
                                               op0=ALU.mult, op1=ALU.add)
                et = spool.tile([128, 4], F32, tag="et", name=f"et{B}")
                nc.scalar.activation(et, z2, AF.Exp)
                nc.vector.tensor_scalar_add(et, et, 1.0)
                rt = spool.tile([128, 4], F32, tag="rt", name=f"rt{B}")
                nc.vector.reciprocal(rt, et)
                nc.vector.tensor_scalar(coeff[:, cs], rt, -2.0, 1.0,
                                        ALU.mult, ALU.add)

            def mv_block(B):
                mvb = pmv.tile([1, 512], F32, tag="pA", name=f"mvb{B}")
                for k in range(KC):
                    if k == 4:
                        warm(1)
                    nc.tensor.matmul(
                        mvb, gwc[:, k:k + 1],
                        wb[:, B * BW + k * 512:B * BW + (k + 1) * 512],
                        start=(k == 0), stop=(k == KC - 1))
                nc.scalar.copy(mv_sb[B], mvb)
                block_tail(B)
                warm(1)

            def meb_chunks(ks):
                for k in ks:
                    ke = slice(k * E, (k + 1) * E)
                    nc.scalar.activation(aef1[:, ke], ef1tp[:, ke],
                                         AF.Copy,
                                         scale=coeff[:, 8 + k:9 + k])
                    nc.tensor.matmul(meb0, aef1[:, k * E:k * E + 128],
                                     ef2tp[:, ke],
                                     start=(k == 0), stop=(k == KC - 1))
                    nc.tensor.matmul(meb1, aef1[:, k * E + 128:(k + 1) * E],
                                     ef2tp[:, ke],
                                     start=(k == 0), stop=(k == KC - 1))

            def an_chunks(ks):
                for k in ks:
                    kn = slice(k * KF, (k + 1) * KF)
                    nc.scalar.activation(a1[:, kn], x1tp[:, kn], AF.Copy,
                                         scale=coeff[:, k:k + 1])
                    nc.tensor.matmul(an, a1[:, kn],
                                     x2tp[:, k * N:(k + 1) * N],
                                     start=(k == 0), stop=(k == KC - 1))

            with FLOOR(0.0110):
                mv_block(0)                  # We rows 0:512
            with FLOOR(0.0120):
                b1_mul(0)
            with FLOOR(0.0133):
                meb_chunks(range(0, 4))
            with FLOOR(0.0150):
                b1_mul(1)
            with FLOOR(0.0166):
                mv_block(2)                  # We rows 512:1024
            with FLOOR(0.0180):
                meb_chunks(range(4, 8))

            # relu(softplus(x) - 0.5): softplus = relu(x) + ln(1+exp(-|x|))
            # All intermediate tiles sliced at the source partition base
            # (compute engines are lane-locked).
            def softplus_relu(src_ap, out_ap, pbase=0):
                p, w = src_ap.shape[0], src_ap.shape[1]
                pe = pbase + p

                def tmp(nm):
                    t = spool.tile([pe, w], F32, tag=f"sp_{nm}",
                                   name=f"sp_{nm}")
                    return t[pbase:pe, :]

                ab_t = tmp("ab")
                nc.scalar.activation(ab_t, src_ap, AF.Abs)
                ex = tmp("ex")
                nc.scalar.activation(ex, ab_t, AF.Exp, scale=-1.0)
                ln = tmp("ln")
                nc.scalar.activation(ln, ex, AF.Ln, bias=1.0)
                rl = tmp("rl")
                nc.scalar.activation(rl, src_ap, AF.Relu)
                pre = tmp("pre")
                nc.vector.scalar_tensor_tensor(out=pre, in0=rl, scalar=-0.5,
                                               in1=ln, op0=ALU.add,
                                               op1=ALU.add)
                nc.vector.tensor_scalar_max(out_ap, pre, 0.0)

            mebs0 = cpool.tile([128, E], BF16, tag="mebs0", name="mebs0")
            mebs1 = cpool.tile([64, E], BF16, tag="mebs1", name="mebs1")
            with FLOOR(0.0190):
                softplus_relu(meb0, mebs0)
                softplus_relu(meb1, mebs1)

            # ---------- McT[a, jc] = Meb[glob(jc), a] (edge compaction) ---
            mct0 = pbig.tile([128, JC], F32, tag="pB", name="mct0")
            mct1 = pbig.tile([64, JC], F32, tag="pC", name="mct1")
            mcts0 = cpool.tile([128, JC], BF16, tag="mcts0", name="mcts0")
            mcts1 = cpool.tile([64, JC], BF16, tag="mcts1", name="mcts1")
            with FLOOR(0.0205):
                nc.tensor.matmul(mct0, mebs0[:, 0:128], selth0,
                                 start=True, stop=False)
                nc.tensor.matmul(mct0, mebs1[:, 0:128], selth1,
                                 start=False, stop=True)
                nc.tensor.matmul(mct1, mebs0[:, 128:192], selth0,
                                 start=True, stop=False)
                nc.tensor.matmul(mct1, mebs1[:, 128:192], selth1,
                                 start=False, stop=True)
                nc.scalar.copy(mcts0, mct0)
                nc.vector.tensor_copy(mcts1, mct1)

            # ---------- Pc rows 0:48 = McT.T @ B1; rows 64:70 = diag ------
            def pc_tiles(tis):
                for ti in tis:
                    t0, t1 = NT[ti]
                    w = t1 - t0
                    pp = ppc.tile([JC, 512], F32, tag="pc", name="pp")
                    nc.tensor.matmul(pp[:, 0:w], mcts0, b1[0][:, t0:t1],
                                     start=True, stop=False)
                    nc.tensor.matmul(pp[:, 0:w], mcts1, b1[1][:, t0:t1],
                                     start=False, stop=True)
                    if ti % 2 == 0:
                        nc.vector.tensor_copy(pc_sb[0:JC, t0:t1], pp[:, 0:w])
                    else:
                        nc.scalar.copy(pc_sb[0:JC, t0:t1], pp[:, 0:w])

            with FLOOR(0.0198):
                mv_block(1)                  # Wn rows 0:512
            with FLOOR(0.0202):
                mv_block(3)                  # Wn rows 512:1024
            with FLOOR(0.0207):
                pc_tiles(range(0, 5))
            with FLOOR(0.0212):
                an_chunks(range(0, 4))
            with FLOOR(0.0216):
                an_chunks(range(4, 8))
            # Mp rows live at an[64:70]; scatter onto the diagonal stride
            msel = cpool.tile([KF, N], BF16, tag="msel", name="msel")
            with FLOOR(0.0222):
                softplus_relu(an[DROW:KF, :], msel[DROW:KF, :], pbase=DROW)
                nc.vector.tensor_copy(pc_sb[DROW:KF, 0:COLS:N + 1],
                                      msel[DROW:KF, :])

            # ---------- final: out_pair = S2c.T @ Pc (diag included) ------
            with FLOOR(0.0232):
                for pa in range(I2P // 2):
                    orow = opool.tile([112, COLS], BF16, tag="orow",
                                      name="orow")
                    for ti, (t0, t1) in enumerate(NT):
                        w = t1 - t0
                        fp = pfin.tile([112, 512], F32, tag="fin", name="fp")
                        nc.tensor.matmul(fp[:, 0:w],
                                         s2ch[:, pa * 112:(pa + 1) * 112],
                                         pc_sb[:, t0:t1],
                                         start=True, stop=True)
                        if ti % 2 == 0:
                            nc.vector.tensor_copy(orow[:, t0:t1], fp[:, 0:w])
                        else:
                            nc.scalar.copy(orow[:, t0:t1], fp[:, 0:w])
                    nc.sync.dma_start(out=d_out[pa * 112:(pa + 1) * 112, :],
                                      in_=orow)

    _split_multiwaits(nc)
    _CACHE["nc"] = nc
    return nc


def _make_in_maps(a):
    bf = ml_dtypes.bfloat16
    gw = a["global_weight"].astype(np.float32)
    gwc = np.ascontiguousarray(gw.reshape(KC, 128).T).astype(bf)

    def wblock(Wfull, r):  # rows 512r:512r+512 -> [128, KC*512] lhs-moving
        sl = Wfull[512 * r:512 * (r + 1), :].astype(np.float32)
        t = sl.T.reshape(KC, 128, 512).transpose(1, 0, 2)
        return t.reshape(128, KC * 512)

    wn, we = a["Wn"], a["We"]
    wbs = {("n", r): wblock(wn, r) for r in range(2)}
    wbs.update({("e", r): wblock(we, r) for r in range(2)})
    wb = np.concatenate([wbs[b] for b in WBLK], axis=1)
    wb = np.ascontiguousarray(wb).astype(bf)

    bnbe = np.zeros((128, 16), np.float32)
    for j in range(16):
        b = a["bn"] if j < 8 else a["be"]
        bnbe[:, j] = 2.0 * b[128 * (j % 8):128 * (j % 8) + 128]

    def chunked(x):  # [n, 1024] -> [128, KC*n] bf16, chunk k = feats 128k+p
        t = x.T.astype(np.float32).reshape(KC, 128, -1).transpose(1, 0, 2)
        return np.ascontiguousarray(t.reshape(128, -1)).astype(bf)

    x2tp = chunked(a["x2"])
    ef1tp = chunked(a["ef1"])
    ef2tp = chunked(a["ef2"])

    ei1 = a["edge_index1"].astype(np.int64)
    ei1f = np.ascontiguousarray(ei1.T).astype(np.float32)
    ei2 = a["edge_index2"].astype(np.int64)

    in_maps = []
    for c in range(N_CORES):
        edges = np.where(ei2[0] // I2P == c)[0]
        assert len(edges) <= JC, f"core {c}: {len(edges)} edges > JC={JC}"
        selth = np.zeros((E, JC), np.float32)
        selth[edges, np.arange(len(edges))] = 1.0
        s2ch = np.zeros((KF, 3 * 112), np.float32)
        for j, e in enumerate(edges):
            i2g = int(ei2[0, e])
            i2l = i2g - I2P * c
            k2r = (int(ei2[1, e]) - i2g) % N
            s2ch[j, (i2l // 2) * 112 + 64 * (i2l % 2) + k2r] = 1.0
        for i2l in range(I2P):  # diag rows select pc_sb rows 64:70
            s2ch[DROW + i2l, (i2l // 2) * 112 + 64 * (i2l % 2)] = 1.0
        # x1 staged into columns 64:70 of the an operand (owned rows only)
        x1own = a["x1"][I2P * c:I2P * (c + 1), :]        # [6, 1024]
        t = x1own.T.astype(np.float32).reshape(KC, 128, I2P)
        x1tp = np.zeros((128, KC * KF), np.float32)
        for k in range(KC):
            x1tp[:, k * KF + DROW:(k + 1) * KF] = t[k]
        in_maps.append({
            "wb": wb,
            "gwc": gwc,
            "bnbe": bnbe,
            "ei1f": ei1f,
            "x1tp": x1tp.astype(bf),
            "x2tp": x2tp,
            "ef1tp": ef1tp,
            "ef2tp": ef2tp,
            "selth": selth.astype(bf),
            "s2ch": s2ch.astype(bf),
        })
    return in_maps


def kernel(**inputs) -> np.ndarray:
    global LAST_RESULTS
    nc = _build()
    a = {k: np.ascontiguousarray(np.asarray(v)) for k, v in inputs.items()}
    in_maps = _make_in_maps(a)
    res = run_bass_kernel_spmd(nc, in_maps, core_ids=list(range(N_CORES)))
    LAST_RESULTS = res

    parts = []
    for c in range(N_CORES):
        # device rows are [pa][0:48] = i2l 2pa, [64:112] = i2l 2pa+1, each
        # [k2rot, (i1, k1)] with k2g = (k2rot + i2l + 6c) mod 48
        o = np.asarray(res.results[c]["out"]).astype(np.float32)
        o = o.reshape(3, 112, COLS)
        o = np.stack([o[i2l // 2, 64 * (i2l % 2):64 * (i2l % 2) + N, :]
                      for i2l in range(I2P)])    # [6, k2rot, 2304]
        o = o.reshape(I2P, N, N, N).transpose(0, 2, 1, 3)
        o = np.stack([np.roll(o[i], i + I2P * c, axis=1)
                      for i in range(I2P)])
        parts.append(o.reshape(ROWS, COLS))
    return np.concatenate(parts, axis=0).astype(np.float32)


if __name__ == "__main__":
    _build()
    print("build OK")


# revision 37
# speedup vs baseline: 1.2160x; 1.0036x over previous
"""Trainium2 Bass kernel for AffinityMatrixConstructLayer (v5).

Math (matching the reference's index conventions):
  weight W[b, a] = softplusrelu( sum_d ef1[b,d]*ce[d]*ef2[a,d] )  (b: g2 edge)
  M[(i2,i1),(k2,k1)] = sum_{b: h2(b)=i2, t2(b)=k2} sum_{a: h1(a)=i1, t1(a)=k1}
                       W[b, a]  +  diag(Mp[i2, i1])
  cn/ce = tanh(Wn/We @ gw + bias)

Design notes:
 - Collectives cost ~55us end-to-end on this stack (measured), so the
   coeff matvec is replicated per core. W streams in FOUR 1MB rho-blocks
   (We rows 0:512, Wn 0:512, We 512:1024, Wn 512:1024) so the matvec,
   tanh and even the Meb GEMM chunks pipeline underneath the 4MB DMA:
   per block, 8 PE matmuls (lhsT = gw chunk [128,1], rhs = W^T chunk,
   N=512) accumulate a [1,512] psum row; 4 PE transposes of [1,128]
   slices + DVE copies build coeff_pre columns; exp-form tanh follows
   per block.  8 dummy matmuls warm the PE HAM clock during the initial
   DMA wait.
 - Each core needs only the <=48 graph-2 edges with head in its 6 block
   rows: one-hot SelT [192,48] compacts Meb -> McT (N=48); then
   Pc = McT.T @ B1 (K=192, N=2304) and out_pair = S2c.T @ Pc x3 pairs.
 - The Mp diagonal is folded into the final GEMM: the an GEMM computes
   the 6 owned Mp rows at partitions 48:54 (host stages x1 there), the
   softplus'd rows are strided-scattered into pc_sb rows 48:54, and
   host-built S2c rows 48:54 select them -- no cross-partition bounce.
 - All big GEMM operands bf16 (f32 PSUM); output written bf16, host
   upcasts (tolerance 2e-2).  ACT funcs stay in one table set.
 - B1 one-hot is built on gpsimd/DVE during the W stream; per-i2 k2
   rotation baked into S2c puts the diagonal at k2rot=0; host un-rotates.
"""

import sys

for _p in ("/opt/trn_rl_repo", "/root/.axon_site/_ro/trn_rl_repo"):
    if _p not in sys.path:
        sys.path.insert(0, _p)

import numpy as np
import ml_dtypes

import concourse.bass as bass
import concourse.mybir as mybir
from concourse.tile import TileContext
from concourse.masks import make_identity
from concourse.bass_utils import run_bass_kernel_spmd

F32 = mybir.dt.float32
BF16 = mybir.dt.bfloat16
I32 = mybir.dt.int32
AF = mybir.ActivationFunctionType
ALU = mybir.AluOpType

N_CORES = 8
N = 48          # nodes per graph
E = 192         # edges per graph
D = 1024        # feature dim
KC = 8          # contraction chunks of 128
I2P = N // N_CORES          # 6 block-rows per core
JC = 48         # compacted graph-2 edge capacity per core
DROW = 64       # diag rows base partition (32-aligned engine access)
KF = DROW + I2P  # final-GEMM K: 48 edges + pad + 6 diag rows
ROWS = I2P * N              # 288 valid output rows per core
OROWS = 3 * 112             # padded device output rows
COLS = N * N                # 2304
NT = [(t * 512, min(COLS, (t + 1) * 512)) for t in range((COLS + 511) // 512)]
# W stream order: block -> (source, row-tile); e-blocks early so the Meb
# path runs under the DMA stream; coeff col of (block, t) = CBASE[B] + t
WBLK = [("e", 0), ("n", 0), ("e", 1), ("n", 1)]
CBASE = [8, 0, 12, 4]

_CACHE: dict = {}
LAST_RESULTS = None


def _split_multiwaits(nc):
    """This walrus build encodes at most one sync-wait per instruction.
    Move extra waits onto injected single-wait drains on the same engine
    (engine queues execute in order, so semantics are preserved)."""
    for f in nc.m.functions:
        for blk in f.blocks:
            out = []
            for inst in blk.instructions:
                si = getattr(inst, "sync_info", None)
                if si is not None and si.on_wait and len(si.on_wait) > 1:
                    waits = list(si.on_wait)
                    for w in waits[:-1]:
                        d = mybir.InstDrain(
                            name=nc.get_next_instruction_name(),
                            ins=[], outs=[], bass_is_fusable=False)
                        d.engine = inst.engine
                        d.sync_info = mybir.SyncInfo(on_wait=[w], on_update=[])
                        out.append(d)
                    si.on_wait = waits[-1:]
                out.append(inst)
            try:
                blk.instructions[:] = out
            except TypeError:
                blk.instructions = out


def _build() -> bass.Bass:
    if "nc" in _CACHE:
        return _CACHE["nc"]
    nc = bass.Bass(trn_type="TRN2", num_devices=N_CORES)

    d_wb = nc.dram_tensor("wb", [128, 4 * KC * 512], BF16,
                          kind="ExternalInput")
    d_gwc = nc.dram_tensor("gwc", [128, KC], BF16, kind="ExternalInput")
    d_bnbe = nc.dram_tensor("bnbe", [128, 16], F32, kind="ExternalInput")
    d_ei1f = nc.dram_tensor("ei1f", [E, 2], F32, kind="ExternalInput")
    d_x1tp = nc.dram_tensor("x1tp", [128, KC * KF], BF16,
                            kind="ExternalInput")
    d_x2tp = nc.dram_tensor("x2tp", [128, KC * N], BF16, kind="ExternalInput")
    d_ef1tp = nc.dram_tensor("ef1tp", [128, KC * E], BF16,
                             kind="ExternalInput")
    d_ef2tp = nc.dram_tensor("ef2tp", [128, KC * E], BF16,
                             kind="ExternalInput")
    d_selth = nc.dram_tensor("selth", [E, JC], BF16, kind="ExternalInput")
    d_s2ch = nc.dram_tensor("s2ch", [KF, 3 * 112], BF16, kind="ExternalInput")
    d_out = nc.dram_tensor("out", [OROWS, COLS], BF16, kind="ExternalOutput")

    with TileContext(nc) as tc:
        with (
            tc.tile_pool(name="const", bufs=1) as cpool,
            tc.tile_pool(name="scratch", bufs=2) as spool,
            tc.tile_pool(name="orow", bufs=3) as opool,
            tc.tile_pool(name="pmv", bufs=1, space="PSUM") as pmv,
            tc.tile_pool(name="pbig", bufs=1, space="PSUM") as pbig,
            tc.tile_pool(name="ppc", bufs=2, space="PSUM") as ppc,
            tc.tile_pool(name="pfin", bufs=2, space="PSUM") as pfin,
        ):
            FLOOR = tc.tile_wait_until   # scheduler virtual-time floor (ms)

            # ---------- front DMA: bandwidth-balanced across the sync
            # (HWDGE, ~150GB/s) and gpsimd (SWDGE, ~250GB/s) queues, in
            # order of real need: e0 W halves + ef chunk-halves first
            # (the Meb critical path), then e1, then the n blocks. ------
            gwc = cpool.tile([128, KC], BF16, tag="gwc", name="gwc")
            nc.sync.dma_start(out=gwc, in_=d_gwc[:, :])
            bnbe = cpool.tile([128, 16], F32, tag="bnbe", name="bnbe")
            nc.sync.dma_start(out=bnbe, in_=d_bnbe[:, :])
            ev_tiles = []
            for lo, hi in ((0, 128), (128, 192)):
                t = cpool.tile([hi - lo, 2], F32, tag=f"ev{lo}",
                               name=f"ev{lo}")
                nc.sync.dma_start(out=t, in_=d_ei1f[lo:hi, :])
                ev_tiles.append(t)
            ident = cpool.tile([128, 128], F32, tag="ident", name="ident")
            make_identity(nc, ident)
            iota48 = cpool.tile([128, N], F32, tag="iota48", name="iota48")
            nc.gpsimd.iota(iota48, pattern=[[1, N]], base=0,
                           channel_multiplier=0,
                           allow_small_or_imprecise_dtypes=True)

            BW = KC * 512
            HWW = BW // 2
            wb = cpool.tile([128, 4 * BW], BF16, tag="wb", name="wb")
            ef1tp = cpool.tile([128, KC * E], BF16, tag="ef1tp", name="ef1tp")
            ef2tp = cpool.tile([128, KC * E], BF16, tag="ef2tp", name="ef2tp")
            x1tp = cpool.tile([128, KC * KF], BF16, tag="x1tp", name="x1tp")
            x2tp = cpool.tile([128, KC * N], BF16, tag="x2tp", name="x2tp")
            selth0 = cpool.tile([128, JC], BF16, tag="selth0", name="selth0")
            selth1 = cpool.tile([64, JC], BF16, tag="selth1", name="selth1")
            s2ch = cpool.tile([KF, 3 * 112], BF16, tag="s2ch", name="s2ch")

            def wh(h, eng):
                eng.dma_start(out=wb[:, h * HWW:(h + 1) * HWW],
                              in_=d_wb[:, h * HWW:(h + 1) * HWW])

            EH = 4 * E
            wh(0, nc.gpsimd)                               # e0 lo
            wh(1, nc.gpsimd)                               # e0 hi
            nc.sync.dma_start(out=ef2tp[:, 0:EH], in_=d_ef2tp[:, 0:EH])
            nc.gpsimd.dma_start(out=ef1tp[:, 0:EH], in_=d_ef1tp[:, 0:EH])
            wh(4, nc.gpsimd)                               # e1 lo
            wh(5, nc.gpsimd)                               # e1 hi
            nc.sync.dma_start(out=ef2tp[:, EH:2 * EH],
                              in_=d_ef2tp[:, EH:2 * EH])
            nc.gpsimd.dma_start(out=ef1tp[:, EH:2 * EH],
                                in_=d_ef1tp[:, EH:2 * EH])
            wh(2, nc.gpsimd)                               # n0 lo
            wh(3, nc.gpsimd)                               # n0 hi
            wh(6, nc.gpsimd)                               # n1 lo
            wh(7, nc.gpsimd)                               # n1 hi
            nc.sync.dma_start(out=x1tp, in_=d_x1tp[:, :])
            nc.sync.dma_start(out=x2tp, in_=d_x2tp[:, :])
            nc.sync.dma_start(out=selth0, in_=d_selth[0:128, :])
            nc.sync.dma_start(out=selth1, in_=d_selth[128:192, :])
            nc.sync.dma_start(out=s2ch, in_=d_s2ch[:, :])

            # pc_sb zero-fill on idle gpsimd
            pc_sb = cpool.tile([KF, COLS], BF16, tag="pc_sb", name="pc_sb")
            nc.gpsimd.memset(pc_sb[32:64, :], 0.0)
            nc.gpsimd.memset(pc_sb[64:KF, :], 0.0)

            # ---------- B1 one-hot built on DVE during the W stream -------
            def incid(col, tag):
                tiles = []
                for ci, p in ((0, 128), (1, 64)):
                    ev = ev_tiles[ci][:, col:col + 1]
                    t = cpool.tile([p, N], BF16, tag=f"{tag}{ci}",
                                   name=f"{tag}{ci}")
                    nc.vector.tensor_tensor(t, iota48[0:p, :],
                                            ev.broadcast_to((p, N)),
                                            ALU.is_equal)
                    tiles.append(t)
                return tiles

            G1T = incid(0, "G1T")
            H1T = incid(1, "H1T")
            b1 = []
            for ci, p in ((0, 128), (1, 64)):
                bt = cpool.tile([p, COLS], BF16, tag=f"b1{ci}",
                                name=f"b1{ci}")
                b1.append(bt)

            def b1_mul(ci):
                p = 128 if ci == 0 else 64
                nc.vector.tensor_mul(
                    b1[ci].rearrange("p (a b) -> p a b", b=N),
                    H1T[ci].unsqueeze(1).broadcast_to((p, N, N)),
                    G1T[ci].unsqueeze(2).broadcast_to((p, N, N)))

            # ---------- PE warmup during the first DMA wait ----------
            dsrc = cpool.tile([128, 512], BF16, tag="dsrc", name="dsrc")
            nc.vector.memset(dsrc, 0.0)
            _wi = [0]

            def warm(n):
                for _ in range(n):
                    dpp = pfin.tile([112, 512], F32, tag="fin",
                                    name=f"warm{_wi[0]}")
                    _wi[0] += 1
                    nc.tensor.matmul(dpp, dsrc[:, 0:112], dsrc,
                                     start=True, stop=True)

            warm(8)

            # ---------- per-block matvec -> coeff columns ----------
            mv_sb = [cpool.tile([1, 512], F32, tag=f"mvsb{B}",
                                name=f"mvsb{B}") for B in range(4)]
            cpre = cpool.tile([128, 16], F32, tag="cpre", name="cpre")
            coeff = cpool.tile([128, 16], F32, tag="coeff", name="coeff")
            aef1 = cpool.tile([128, KC * E], BF16, tag="aef1", name="aef1")
            a1 = cpool.tile([128, KC * KF], BF16, tag="a1", name="a1")
            meb0 = pbig.tile([128, E], F32, tag="pB", name="meb0")
            meb1 = pbig.tile([64, E], F32, tag="pC", name="meb1")
            an = pmv.tile([KF, N], F32, tag="pAn", name="an")

            def block_tail(B):
                """mv psum row -> coeff cols CBASE[B]..CBASE[B]+4."""
                base = CBASE[B]
                pt = ppc.tile([128, 16], F32, tag="pc", name=f"pt{B}")
                for t in range(4):
                    nc.tensor.transpose(
                        pt[:, base + t:base + t + 1],
                        mv_sb[B][:, t * 128:(t + 1) * 128],
                        ident[0:1, 0:1])
                cs = slice(base, base + 4)
                nc.vector.tensor_copy(cpre[:, cs], pt[:, cs])
                z2 = spool.tile([128, 4], F32, tag="z2", name=f"z2{B}")
                nc.vector.scalar_tensor_tensor(out=z2, in0=cpre[:, cs],
                                               scalar=2.0, in1=bnbe[:, cs],
                                               op0=ALU.mult, op1=ALU.add)
                et = spool.tile([128, 4], F32, tag="et", name=f"et{B}")
                nc.scalar.activation(et, z2, AF.Exp)
                nc.vector.tensor_scalar_add(et, et, 1.0)
                rt = spool.tile([128, 4], F32, tag="rt", name=f"rt{B}")
                nc.vector.reciprocal(rt, et)
                nc.vector.tensor_scalar(coeff[:, cs], rt, -2.0, 1.0,
                                        ALU.mult, ALU.add)

            def mv_block(B):
                mvb = pmv.tile([1, 512], F32, tag="pA", name=f"mvb{B}")
                for k in range(KC):
                    if k == 4:
                        warm(1)
                    nc.tensor.matmul(
                        mvb, gwc[:, k:k + 1],
                        wb[:, B * BW + k * 512:B * BW + (k + 1) * 512],
                        start=(k == 0), stop=(k == KC - 1))
                nc.scalar.copy(mv_sb[B], mvb)
                block_tail(B)
                warm(1)

            def meb_chunks(ks):
                for k in ks:
                    ke = slice(k * E, (k + 1) * E)
                    nc.scalar.activation(aef1[:, ke], ef1tp[:, ke],
                                         AF.Copy,
                                         scale=coeff[:, 8 + k:9 + k])
                    nc.tensor.matmul(meb0, aef1[:, k * E:k * E + 128],
                                     ef2tp[:, ke],
                                     start=(k == 0), stop=(k == KC - 1))
                    nc.tensor.matmul(meb1, aef1[:, k * E + 128:(k + 1) * E],
                                     ef2tp[:, ke],
                                     start=(k == 0), stop=(k == KC - 1))

            def an_chunks(ks):
                for k in ks:
                    kn = slice(k * KF, (k + 1) * KF)
                    nc.scalar.activation(a1[:, kn], x1tp[:, kn], AF.Copy,
                                         scale=coeff[:, k:k + 1])
                    nc.tensor.matmul(an, a1[:, kn],
                                     x2tp[:, k * N:(k + 1) * N],
                                     start=(k == 0), stop=(k == KC - 1))

            with FLOOR(0.0115):
                mv_block(0)                  # We rows 0:512
            with FLOOR(0.0125):
                b1_mul(0)
            with FLOOR(0.0132):
                meb_chunks(range(0, 4))
            with FLOOR(0.0150):
                b1_mul(1)
            with FLOOR(0.0167):
                mv_block(2)                  # We rows 512:1024
            with FLOOR(0.0178):
                meb_chunks(range(4, 8))

            # relu(softplus(x) - 0.5): softplus = relu(x) + ln(1+exp(-|x|))
            # All intermediate tiles sliced at the source partition base
            # (compute engines are lane-locked).
            def softplus_relu(src_ap, out_ap, pbase=0):
                p, w = src_ap.shape[0], src_ap.shape[1]
                pe = pbase + p

                def tmp(nm):
                    t = spool.tile([pe, w], F32, tag=f"sp_{nm}",
                                   name=f"sp_{nm}")
                    return t[pbase:pe, :]

                ab_t = tmp("ab")
                nc.scalar.activation(ab_t, src_ap, AF.Abs)
                ex = tmp("ex")
                nc.scalar.activation(ex, ab_t, AF.Exp, scale=-1.0)
                ln = tmp("ln")
                nc.scalar.activation(ln, ex, AF.Ln, bias=1.0)
                rl = tmp("rl")
                nc.scalar.activation(rl, src_ap, AF.Relu)
                pre = tmp("pre")
                nc.vector.scalar_tensor_tensor(out=pre, in0=rl, scalar=-0.5,
                                               in1=ln, op0=ALU.add,
                                               op1=ALU.add)
                nc.vector.tensor_scalar_max(out_ap, pre, 0.0)

            mebs0 = cpool.tile([128, E], BF16, tag="mebs0", name="mebs0")
            mebs1 = cpool.tile([64, E], BF16, tag="mebs1", name="mebs1")
            with FLOOR(0.0190):
                softplus_relu(meb0, mebs0)
                softplus_relu(meb1, mebs1)

            # ---------- McT[a, jc] = Meb[glob(jc), a] (edge compaction) ---
            mct0 = pbig.tile([128, JC], F32, tag="pB", name="mct0")
            mct1 = pbig.tile([64, JC], F32, tag="pC", name="mct1")
            mcts0 = cpool.tile([128, JC], BF16, tag="mcts0", name="mcts0")
            mcts1 = cpool.tile([64, JC], BF16, tag="mcts1", name="mcts1")
            with FLOOR(0.0200):
                nc.tensor.matmul(mct0, mebs0[:, 0:128], selth0,
                                 start=True, stop=False)
                nc.tensor.matmul(mct0, mebs1[:, 0:128], selth1,
                                 start=False, stop=True)
                nc.tensor.matmul(mct1, mebs0[:, 128:192], selth0,
                                 start=True, stop=False)
                nc.tensor.matmul(mct1, mebs1[:, 128:192], selth1,
                                 start=False, stop=True)
                nc.scalar.copy(mcts0, mct0)
                nc.vector.tensor_copy(mcts1, mct1)

            # ---------- Pc rows 0:48 = McT.T @ B1; rows 64:70 = diag ------
            def pc_tiles(tis):
                for ti in tis:
                    t0, t1 = NT[ti]
                    w = t1 - t0
                    pp = ppc.tile([JC, 512], F32, tag="pc", name="pp")
                    nc.tensor.matmul(pp[:, 0:w], mcts0, b1[0][:, t0:t1],
                                     start=True, stop=False)
                    nc.tensor.matmul(pp[:, 0:w], mcts1, b1[1][:, t0:t1],
                                     start=False, stop=True)
                    if ti % 2 == 0:
                        nc.vector.tensor_copy(pc_sb[0:JC, t0:t1], pp[:, 0:w])
                    else:
                        nc.scalar.copy(pc_sb[0:JC, t0:t1], pp[:, 0:w])

            with FLOOR(0.0192):
                mv_block(1)                  # Wn rows 0:512
            with FLOOR(0.0224):
                mv_block(3)                  # Wn rows 512:1024
            with FLOOR(0.0202):
                pc_tiles(range(0, 5))
            with FLOOR(0.0204):
                an_chunks(range(0, 4))
            with FLOOR(0.0238):
                an_chunks(range(4, 8))
            # Mp rows live at an[64:70]; scatter onto the diagonal stride
            msel = cpool.tile([KF, N], BF16, tag="msel", name="msel")
            with FLOOR(0.0242):
                warm(3)
                softplus_relu(an[DROW:KF, :], msel[DROW:KF, :], pbase=DROW)
                nc.vector.tensor_copy(pc_sb[DROW:KF, 0:COLS:N + 1],
                                      msel[DROW:KF, :])

            # ---------- final: out_pair = S2c.T @ Pc (diag included) ------
            with FLOOR(0.0250):
                for pa in range(I2P // 2):
                    orow = opool.tile([112, COLS], BF16, tag="orow",
                                      name="orow")
                    for ti, (t0, t1) in enumerate(NT):
                        w = t1 - t0
                        fp = pfin.tile([112, 512], F32, tag="fin", name="fp")
                        nc.tensor.matmul(fp[:, 0:w],
                                         s2ch[:, pa * 112:(pa + 1) * 112],
                                         pc_sb[:, t0:t1],
                                         start=True, stop=True)
                        if ti % 2 == 0:
                            nc.vector.tensor_copy(orow[:, t0:t1], fp[:, 0:w])
                        else:
                            nc.scalar.copy(orow[:, t0:t1], fp[:, 0:w])
                    nc.sync.dma_start(out=d_out[pa * 112:(pa + 1) * 112, :],
                                      in_=orow)

    _split_multiwaits(nc)
    _CACHE["nc"] = nc
    return nc


def _make_in_maps(a):
    bf = ml_dtypes.bfloat16
    gw = a["global_weight"].astype(np.float32)
    gwc = np.ascontiguousarray(gw.reshape(KC, 128).T).astype(bf)

    def wblock(Wfull, r):  # rows 512r:512r+512 -> [128, KC*512] lhs-moving
        sl = Wfull[512 * r:512 * (r + 1), :].astype(np.float32)
        t = sl.T.reshape(KC, 128, 512).transpose(1, 0, 2)
        return t.reshape(128, KC * 512)

    wn, we = a["Wn"], a["We"]
    wbs = {("n", r): wblock(wn, r) for r in range(2)}
    wbs.update({("e", r): wblock(we, r) for r in range(2)})
    wb = np.concatenate([wbs[b] for b in WBLK], axis=1)
    wb = np.ascontiguousarray(wb).astype(bf)

    bnbe = np.zeros((128, 16), np.float32)
    for j in range(16):
        b = a["bn"] if j < 8 else a["be"]
        bnbe[:, j] = 2.0 * b[128 * (j % 8):128 * (j % 8) + 128]

    def chunked(x):  # [n, 1024] -> [128, KC*n] bf16, chunk k = feats 128k+p
        t = x.T.astype(np.float32).reshape(KC, 128, -1).transpose(1, 0, 2)
        return np.ascontiguousarray(t.reshape(128, -1)).astype(bf)

    x2tp = chunked(a["x2"])
    ef1tp = chunked(a["ef1"])
    ef2tp = chunked(a["ef2"])

    ei1 = a["edge_index1"].astype(np.int64)
    ei1f = np.ascontiguousarray(ei1.T).astype(np.float32)
    ei2 = a["edge_index2"].astype(np.int64)

    in_maps = []
    for c in range(N_CORES):
        edges = np.where(ei2[0] // I2P == c)[0]
        assert len(edges) <= JC, f"core {c}: {len(edges)} edges > JC={JC}"
        selth = np.zeros((E, JC), np.float32)
        selth[edges, np.arange(len(edges))] = 1.0
        s2ch = np.zeros((KF, 3 * 112), np.float32)
        for j, e in enumerate(edges):
            i2g = int(ei2[0, e])
            i2l = i2g - I2P * c
            k2r = (int(ei2[1, e]) - i2g) % N
            s2ch[j, (i2l // 2) * 112 + 64 * (i2l % 2) + k2r] = 1.0
        for i2l in range(I2P):  # diag rows select pc_sb rows 64:70
            s2ch[DROW + i2l, (i2l // 2) * 112 + 64 * (i2l % 2)] = 1.0
        # x1 staged into columns 64:70 of the an operand (owned rows only)
        x1own = a["x1"][I2P * c:I2P * (c + 1), :]        # [6, 1024]
        t = x1own.T.astype(np.float32).reshape(KC, 128, I2P)
        x1tp = np.zeros((128, KC * KF), np.float32)
        for k in range(KC):
            x1tp[:, k * KF + DROW:(k + 1) * KF] = t[k]
        in_maps.append({
            "wb": wb,
            "gwc": gwc,
            "bnbe": bnbe,
            "ei1f": ei1f,
            "x1tp": x1tp.astype(bf),
            "x2tp": x2tp,
            "ef1tp": ef1tp,
            "ef2tp": ef2tp,
            "selth": selth.astype(bf),
            "s2ch": s2ch.astype(bf),
        })
    return in_maps


def kernel(**inputs) -> np.ndarray:
    global LAST_RESULTS
    nc = _build()
    a = {k: np.ascontiguousarray(np.asarray(v)) for k, v in inputs.items()}
    in_maps = _make_in_maps(a)
    res = run_bass_kernel_spmd(nc, in_maps, core_ids=list(range(N_CORES)))
    LAST_RESULTS = res

    parts = []
    for c in range(N_CORES):
        # device rows are [pa][0:48] = i2l 2pa, [64:112] = i2l 2pa+1, each
        # [k2rot, (i1, k1)] with k2g = (k2rot + i2l + 6c) mod 48
        o = np.asarray(res.results[c]["out"]).astype(np.float32)
        o = o.reshape(3, 112, COLS)
        o = np.stack([o[i2l // 2, 64 * (i2l % 2):64 * (i2l % 2) + N, :]
                      for i2l in range(I2P)])    # [6, k2rot, 2304]
        o = o.reshape(I2P, N, N, N).transpose(0, 2, 1, 3)
        o = np.stack([np.roll(o[i], i + I2P * c, axis=1)
                      for i in range(I2P)])
        parts.append(o.reshape(ROWS, COLS))
    return np.concatenate(parts, axis=0).astype(np.float32)


if __name__ == "__main__":
    _build()
    print("build OK")
